# revision 1
# baseline (speedup 1.0000x reference)
"""Trainium2 Bass kernel for nn_DenseAttnProcessor (sparse_attention).

Cross-attention block: q = hs@Wq, k/v = ehs@{Wk,Wv}, per-head softmax((q k^T)/8
+ col_bias) @ v, @Wo + bo + residual.  B=8 batches -> data-parallel, one batch
per NeuronCore (no collectives).

Per-core dataflow (everything lives in "transposed" orientation so that every
matmul contraction has its operand already partition-major; softmax runs on
scoresT [T, q] with the per-head denominator handled by a ones-column matmul, a
reciprocal, and a K=1 broadcast matmul):

  stage A (once):  ehsT (host-pretransposed, bf16) -> k,v [77,1024] via matmul
                   -> kT via PE transpose -> M_h = v_h @ Wo_h [77,1024];
                   M rows DMA-packed into a [16*77+1, 1024] stack; the +bo
                   term rides as an extra stack row paired with an all-ones
                   probs row.
  stage B (8 chunks of 512 q rows):
                   hs chunk f32 -> bf16 cast -> XBAR DMA-transpose -> hsT [C, q]
                   qT = Wq^T@hsT (psum accum over C) [inner, q]
                   per head: scoresT [77,512] = kT_h^T qT_h; z = Exp(scoresT)
                   * exp(col_bias)^T (host-precomputed multiplicative mask,
                   exact "set-column" suppression semantics, rows without
                   suppression are exactly 1.0); D = ones^T z; Dinv via fast
                   DVE reciprocal; DinvB via K=1 broadcast matmul; probsT =
                   z * DinvB, DMA-packed into the [16*77+1, 512] stack;
                   out[q,C] = sum_kt probsT_kt^T @ M_kt (10 K=128 matmuls)
                   + residual (f32) -> DMA out.  Chunks are software-
                   pipelined: softmax(ci-1) is emitted interleaved with
                   qT(ci) so the PE stream stays dense (HAM stays warm).

Inputs are the full unsharded arrays as produced by setup_inputs(); host side
only shards/casts/transposes small tensors and computes the tiny [2,77]/[2,4096]
suppression vectors.
"""

import sys

for _p in ("/opt/trn_rl_repo",):
    if _p not in sys.path:
        sys.path.insert(0, _p)

import numpy as np
import ml_dtypes

import concourse.mybir as mybir
import concourse.tile as tile
from concourse import bacc
from concourse.bass import ds
from concourse.masks import make_identity

F32 = mybir.dt.float32
BF16 = mybir.dt.bfloat16
AF = mybir.ActivationFunctionType

B, HW, C, CT, T, H, D = 8, 4096, 1024, 2048, 77, 16, 64
SUPPRESS = 20.0
RT = H * T + 1                # 1233 stacked rows (16*77 head rows + bo row)
NKT = (RT + 127) // 128       # 10 K-tiles for the AV matmul
NQ = 512                      # q rows per chunk
NCHUNK = HW // NQ             # 8
BO_TILE, BO_PART = (H * T) // 128, (H * T) % 128   # bo/ones row: tile 9, p 80


def _pack_pieces(h):
    """DMA pieces for packing head h's 77 rows at stacked row 77*h, split at
    128-row tile boundaries.  Returns list of (tile_idx, part_base, src_start,
    nrows).  (DMA writes have no partition-alignment restrictions.)"""
    g = T * h
    pieces = []
    pos = 0
    while pos < T:
        gg = g + pos
        ti, d = gg // 128, gg % 128
        n = min(T - pos, 128 - d)
        pieces.append((ti, d, pos, n))
        pos += n
    return pieces


def build_nc():
    nc = bacc.Bacc("TRN2", target_bir_lowering=False, debug=False)

    hs = nc.dram_tensor("hs", [HW, C], F32, kind="ExternalInput")
    ehsT = nc.dram_tensor("ehsT", [CT, T], BF16, kind="ExternalInput")
    wq = nc.dram_tensor("wq", [C, C], BF16, kind="ExternalInput")
    wk = nc.dram_tensor("wk", [CT, C], BF16, kind="ExternalInput")
    wv = nc.dram_tensor("wv", [CT, C], BF16, kind="ExternalInput")
    wo = nc.dram_tensor("wo", [C, C], BF16, kind="ExternalInput")
    euabt = nc.dram_tensor("euabt", [T, HW], BF16, kind="ExternalInput")
    bo = nc.dram_tensor("bo", [1, C], BF16, kind="ExternalInput")
    out = nc.dram_tensor("out", [HW, C], F32, kind="ExternalOutput")

    with tile.TileContext(nc) as tc:
        with (
            tc.tile_pool(name="const", bufs=1) as const,
            tc.tile_pool(name="persist", bufs=1) as persist,
        ):
            ident = const.tile([128, 128], BF16)
            make_identity(nc, ident)
            ones_col = const.tile([T, 1], BF16)
            nc.any.memset(ones_col, 1.0)
            ones_row = const.tile([1, T], BF16)
            nc.any.memset(ones_row, 1.0)
            ones_q = const.tile([1, NQ], BF16)
            nc.any.memset(ones_q, 1.0)
            eu_sb = const.tile([T, HW], BF16)
            nc.sync.dma_start(eu_sb, euabt[:, :])

            # persistent stacks
            kT_sb = persist.tile([128, C // 128, T], BF16)        # [inner, t]
            m_tiles = [persist.tile([128, C], BF16, name=f"m{i}") for i in range(NKT)]
            prob_bufs = [
                [persist.tile([128, NQ], BF16, name=f"pb{b}_{i}") for i in range(NKT)]
                for b in range(2)
            ]
            wq_tiles = [persist.tile([128, C], BF16, name=f"wqt{i}") for i in range(C // 128)]
            for i in range(C // 128):
                nc.sync.dma_start(wq_tiles[i], wq[ds(128 * i, 128), :])

            # only the last stack tile has rows past the packed head rows;
            # zero it so the AV matmuls see zeros there, then land bo/ones.
            nc.any.memset(m_tiles[BO_TILE], 0.0)
            for bset in prob_bufs:
                nc.any.memset(bset[BO_TILE], 0.0)
                nc.sync.dma_start(
                    bset[BO_TILE][BO_PART : BO_PART + 1, :], ones_q
                )
            nc.sync.dma_start(m_tiles[BO_TILE][BO_PART : BO_PART + 1, :], bo[:, :])

            # ---------------- stage A: k, v, kT, M ----------------
            with (
                tc.tile_pool(name="sa_sb", bufs=3) as sa_sb,
                tc.tile_pool(name="sa_w", bufs=3) as sa_w,
                tc.tile_pool(name="sa_ps", bufs=2, space="PSUM") as sa_ps,
            ):
                ehsT_sb = sa_sb.tile([128, CT // 128, T], BF16, bufs=1)
                for j in range(CT // 128):
                    nc.sync.dma_start(ehsT_sb[:, j, :], ehsT[ds(128 * j, 128), :])

                kv_sb = {}
                for name, wten in (("k", wk), ("v", wv)):
                    kv_ps = sa_ps.tile([T, C], F32, tag="kvps", bufs=1)
                    for j in range(CT // 128):
                        wt = sa_w.tile([128, C], BF16, tag="wkv")
                        nc.sync.dma_start(wt, wten[ds(128 * j, 128), :])
                        for nh in range(2):
                            nc.tensor.matmul(
                                kv_ps[:, ds(512 * nh, 512)],
                                ehsT_sb[:, j, :],
                                wt[:, ds(512 * nh, 512)],
                                start=(j == 0),
                                stop=(j == CT // 128 - 1),
                            )
                    kvs = sa_sb.tile([T, C], BF16, tag=f"{name}sb", bufs=1)
                    nc.any.tensor_copy(kvs, kv_ps)
                    kv_sb[name] = kvs

                # kT / vT via PE transpose of 128-column slices
                vT_sb = sa_sb.tile([128, C // 128, T], BF16, bufs=1)
                for src, dst in ((kv_sb["k"], kT_sb), (kv_sb["v"], vT_sb)):
                    for i in range(C // 128):
                        tp = sa_ps.tile([128, T], BF16, tag="tpa")
                        nc.tensor.transpose(tp, src[:, ds(128 * i, 128)], ident[:T, :T])
                        nc.any.tensor_copy(dst[:, i, :], tp)

                # M_h = v_h @ Wo_h, packed at stacked row 96h (+ bo at row 95)
                wot = None
                for h in range(H):
                    i, po = h // 2, (h % 2) * 64
                    if h % 2 == 0:
                        wot = sa_w.tile([128, C], BF16, tag="wot")
                        nc.sync.dma_start(wot, wo[ds(128 * i, 128), :])
                    m_ps = sa_ps.tile([T, C], F32, tag="mps")
                    for nh in range(2):
                        nc.tensor.matmul(
                            m_ps[:, ds(512 * nh, 512)],
                            vT_sb[ds(po, 64), i, :],
                            wot[ds(po, 64), ds(512 * nh, 512)],
                            start=True,
                            stop=True,
                        )
                    m_stg = sa_sb.tile([T, C], BF16, tag="mstg")
                    nc.any.tensor_copy(m_stg, m_ps)
                    for (ti, pb, s0, nr) in _pack_pieces(h):
                        nc.gpsimd.dma_start(
                            m_tiles[ti][ds(pb, nr), :], m_stg[ds(s0, nr), :]
                        )

            # ---------------- stage B: software-pipelined q chunks ----------------
            # Engine streams execute in emission order, so softmax(ci-1) head
            # chains are interleaved with qT(ci) matmul groups at build time:
            # the PE stream then always has dense matmul work queued and the
            # HAM clock gate stays open.
            with (
                tc.tile_pool(name="hsp", bufs=2) as hsp,
                tc.tile_pool(name="work", bufs=2) as work,
                tc.tile_pool(name="soft", bufs=4) as soft,
                tc.tile_pool(name="ops", bufs=2, space="PSUM") as ops,
            ):
                st = {}

                def load(ci):
                    q0 = NQ * ci
                    hs_f = hsp.tile([128, NQ // 128, C], F32, tag="hsf")
                    for qj in range(NQ // 128):
                        nc.sync.dma_start(
                            hs_f[:, qj, :], hs[ds(q0 + 128 * qj, 128), :]
                        )
                    hs_bf = work.tile([128, NQ // 128, C], BF16, tag="hsbf")
                    for qj in range(NQ // 128):
                        nc.scalar.copy(hs_bf[:, qj, :], hs_f[:, qj, :])
                    hsT = work.tile([128, C // 128, NQ], BF16, tag="hsT")
                    for qj in range(NQ // 128):
                        nc.sync.dma_start(
                            hsT[:, :, ds(128 * qj, 128)],
                            hs_bf[:, qj, :],
                            transpose=True,
                        )
                    qT = work.tile([128, C // 128, NQ], BF16, tag="qT")
                    st[ci] = dict(hs_f=hs_f, hsT=hsT, qT=qT)

                def qt_group(ci, ij):
                    hsT, qT = st[ci]["hsT"], st[ci]["qT"]
                    q_ps = ops.tile([128, NQ], F32, tag="qps", bufs=1)
                    for cj in range(C // 128):
                        nc.tensor.matmul(
                            q_ps,
                            wq_tiles[cj][:, ds(128 * ij, 128)],
                            hsT[:, cj, :],
                            start=(cj == 0),
                            stop=(cj == C // 128 - 1),
                        )
                    nc.any.tensor_copy(qT[:, ij, :], q_ps)

                def sm_head1(ci, h):
                    q0 = NQ * ci
                    qT = st[ci]["qT"]
                    i, po = h // 2, (h % 2) * 64
                    sT_ps = ops.tile([T, NQ], F32, tag="sT", bufs=2)
                    nc.tensor.matmul(
                        sT_ps,
                        kT_sb[ds(po, 64), i, :],
                        qT[ds(po, 64), i, :],
                        start=True,
                        stop=True,
                    )
                    expT = soft.tile([T, NQ], BF16, tag="expT", bufs=4)
                    nc.scalar.activation(expT, sT_ps, AF.Exp)
                    # multiplicative suppression mask exp(col_bias^T), host-
                    # precomputed; rows without suppression are exactly 1.0
                    z = soft.tile([T, NQ], BF16, tag="z", bufs=16, name=f"z{h}")
                    nc.vector.tensor_mul(z, expT, eu_sb[:, ds(q0, NQ)])
                    st[ci].setdefault("z", {})[h] = z

                def emit_d(ci, h):
                    d_ps = ops.tile([1, NQ], F32, tag="dps", bufs=2, name=f"dps{h}")
                    nc.tensor.matmul(d_ps, ones_col, st[ci]["z"][h], start=True, stop=True)
                    return d_ps

                def sm_head2(ci, h, d_ps, d_next):
                    prob = prob_bufs[ci % 2]
                    z = st[ci]["z"][h]
                    dinv = soft.tile([1, NQ], F32, tag="dinv", bufs=2)
                    nc.vector.reciprocal_approx_fast(dinv, d_ps)
                    dinv_bf = soft.tile([1, NQ], BF16, tag="dinvbf", bufs=2)
                    nc.scalar.copy(dinv_bf, dinv)
                    nxt = emit_d(ci, h + 1) if d_next else None
                    db_ps = ops.tile([T, NQ], F32, tag="db", bufs=1)
                    nc.tensor.matmul(db_ps, ones_row, dinv_bf, start=True, stop=True)
                    p_stg = soft.tile([T, NQ], BF16, tag="pstg", bufs=4)
                    nc.vector.tensor_mul(p_stg, z, db_ps)
                    for (ti, pb, s0, nr) in _pack_pieces(h):
                        nc.sync.dma_start(
                            prob[ti][ds(pb, nr), :], p_stg[ds(s0, nr), :]
                        )
                    return nxt

                def av(ci):
                    q0 = NQ * ci
                    prob = prob_bufs[ci % 2]
                    hs_f = st[ci]["hs_f"]
                    for qj in range(NQ // 128):
                        for nh in range(2):
                            o_ps = ops.tile([128, 512], F32, tag="ops", bufs=2)
                            for kt in range(NKT):
                                nc.tensor.matmul(
                                    o_ps,
                                    prob[kt][:, ds(128 * qj, 128)],
                                    m_tiles[kt][:, ds(512 * nh, 512)],
                                    start=(kt == 0),
                                    stop=(kt == NKT - 1),
                                )
                            o_sb = work.tile([128, 512], F32, tag="osb", bufs=3)
                            nc.vector.tensor_add(
                                o_sb, o_ps, hs_f[:, qj, ds(512 * nh, 512)]
                            )
                            nc.sync.dma_start(
                                out[ds(q0 + 128 * qj, 128), ds(512 * nh, 512)],
                                o_sb,
                            )

                load(0)
                for ij in range(C // 128):
                    qt_group(0, ij)
                for ci in range(1, NCHUNK + 1):
                    if ci < NCHUNK:
                        load(ci)
                    for h in range(H):
                        sm_head1(ci - 1, h)
                        if ci < NCHUNK and h % 2 == 0:
                            qt_group(ci, h // 2)
                    d_cur = emit_d(ci - 1, 0)
                    for h in range(H):
                        d_cur = sm_head2(ci - 1, h, d_cur, h + 1 < H)
                    av(ci - 1)

    nc.compile()
    return nc


_NC_CACHE = {}


def get_nc():
    if "nc" not in _NC_CACHE:
        _NC_CACHE["nc"] = build_nc()
    return _NC_CACHE["nc"]


def _bf16(x):
    return np.asarray(x, dtype=ml_dtypes.bfloat16)


def make_in_maps(inputs):
    hs = np.ascontiguousarray(np.asarray(inputs["hidden_states"], dtype=np.float32))
    ehs = np.asarray(inputs["encoder_hidden_states"], dtype=np.float32)
    mask_A = np.asarray(inputs["mask_A"], dtype=np.float32)
    mask_B = np.asarray(inputs["mask_B"], dtype=np.float32)
    Wq = np.asarray(inputs["Wq"], dtype=np.float32)
    Wk = np.asarray(inputs["Wk"], dtype=np.float32)
    Wv = np.asarray(inputs["Wv"], dtype=np.float32)
    Wo = np.asarray(inputs["Wo"], dtype=np.float32)
    bo = np.asarray(inputs["bo"], dtype=np.float32)
    idxA = np.asarray(inputs["token_indices_A"]).astype(np.int64) % T
    idxB = np.asarray(inputs["token_indices_B"]).astype(np.int64) % T

    # suppression as a multiplicative mask: exp(col_bias)^T [77, HW].
    # col_bias "set" semantics: B overwrites A; rows not in A|B are exactly 1.
    col_bias = np.zeros((HW, T), np.float32)
    col_bias[:, idxA] = (-SUPPRESS * (1.0 - mask_A))[:, None]
    col_bias[:, idxB] = (-SUPPRESS * (1.0 - mask_B))[:, None]
    euabt = np.exp(col_bias.T)

    scale = 1.0 / np.sqrt(D)
    wq_bf = _bf16(Wq * scale)
    wk_bf, wv_bf, wo_bf = _bf16(Wk), _bf16(Wv), _bf16(Wo)
    euabt_bf = _bf16(euabt)
    bo_bf = _bf16(bo[None, :])

    in_maps = []
    for b in range(B):
        in_maps.append(
            {
                "hs": hs[b],
                "ehsT": _bf16(ehs[b].T.copy()),
                "wq": wq_bf,
                "wk": wk_bf,
                "wv": wv_bf,
                "wo": wo_bf,
                "euabt": euabt_bf,
                "bo": bo_bf,
            }
        )
    return in_maps


def kernel(**inputs) -> np.ndarray:
    from concourse.bass_utils import run_bass_kernel_spmd

    nc = get_nc()
    in_maps = make_in_maps(inputs)
    res = run_bass_kernel_spmd(nc, in_maps, core_ids=list(range(B)))
    return np.stack([res.results[b]["out"] for b in range(B)]).astype(np.float32)



# revision 5
# speedup vs baseline: 1.2268x; 1.2268x over previous
"""Trainium2 Bass kernel for nn_DenseAttnProcessor (sparse_attention).

Cross-attention block: q = hs@Wq, k/v = ehs@{Wk,Wv}, per-head
softmax((q k^T)/8 + col_bias) @ v, @Wo + bo + residual.  B=8 batches ->
data-parallel, one batch per NeuronCore (no collectives).

v2 design notes (vs the bf16 baseline):
  * fp8e4 DoubleRow matmuls (K=256 per instruction) for the three big GEMMs:
    qT = Wq^T hsT, k/v = Wk/Wv^T ehsT, and the fused AV out = probsT^T @ M
    (M = v@Wo stacked per head).  Host pre-quantizes/pre-pairs operands.
  * suppression col_bias rides the scores matmul as 2 extra contraction rows
    (K=66): kTaug rows 64/65 = one-hot u_A/u_B over key tokens, qTaug rows
    64/65 = (SQ*SK)*(-20)*(1-mask) per query.  "Set" semantics (B overwrites
    A) is host-encoded in u_A.  No mask multiply on DVE.
  * softmax in the transposed [T, q] orientation: D via K=77 ones-matmul
    pairs into PSUM partitions {0,64}; one batched DVE reciprocal per pair;
    GpSimd partition_broadcast of 1/D; one DVE mul -> fp8 probs (x SP).
  * probs x SP(=4) and M / SP so the AV PSUM is the unscaled attention
    output: epilogue is a single DVE residual add, then DMA out.
  * All steady-state DMAs are triggered from GpSimd (cheap dispatch).
  * Software pipeline per iteration ci: scores/softmax(ci), qT(ci+1),
    AV(ci-1) interleaved in groups of 2 heads so the PE stream stays dense.
"""

import sys

for _p in ("/opt/trn_rl_repo",):
    if _p not in sys.path:
        sys.path.insert(0, _p)

import numpy as np
import ml_dtypes

import concourse.mybir as mybir
import concourse.tile as tile
from concourse import bacc
from concourse.bass import ds
from concourse.masks import make_identity

F32 = mybir.dt.float32
BF16 = mybir.dt.bfloat16
F8 = mybir.dt.float8e4
AF = mybir.ActivationFunctionType
DR = mybir.MatmulPerfMode.DoubleRow

B, HW, C, CT, T, H, D = 8, 4096, 1024, 2048, 77, 16, 64
SUPPRESS = 20.0
NQ = 512                      # q rows per chunk
NCHUNK = HW // NQ             # 8
SQ = 32.0                     # q-side scale folded into Wq (with 1/sqrt(D))
SK = 32.0                     # k-side scale folded into Wk/Wv
SP = 4.0                      # probs scale (probs*SP, M/SP)
ESCALE = 1.0 / (SQ * SK)      # scores psum = SQ*SK*logits
NKT = 5                       # AV K-tile pairs (stack rows 16*77+1 -> 1280)
TPAD = 80                     # ehsT free-dim pad (fp8 DR needs step%16==0)
BO_KT, BO_I, BO_P = 4, 1, 80  # stack row 1232 = 256*4 + 128*1 + 80


def _stack_pieces(g0, nrows):
    """Split stacked rows [g0, g0+nrows) at 128 boundaries.
    Returns [(kt, i, p, s0, n)] with stack row g = 256*kt + 128*i + p."""
    pieces = []
    pos = 0
    while pos < nrows:
        g = g0 + pos
        kt, r = g // 256, g % 256
        i, p = r // 128, r % 128
        n = min(nrows - pos, 128 - p)
        pieces.append((kt, i, p, pos, n))
        pos += n
    return pieces


def build_nc():
    nc = bacc.Bacc("TRN2", target_bir_lowering=False, debug=False)

    hsT8 = nc.dram_tensor("hsT8", [4, 128, 2, HW], F8, kind="ExternalInput")
    hres = nc.dram_tensor("hres", [HW, C], BF16, kind="ExternalInput")
    wq8 = nc.dram_tensor("wq8", [4, 128, 2, C], F8, kind="ExternalInput")
    ehsT8 = nc.dram_tensor("ehsT8", [8, 128, 2, TPAD], F8, kind="ExternalInput")
    wk8 = nc.dram_tensor("wk8", [8, 128, 2, C], F8, kind="ExternalInput")
    wv8 = nc.dram_tensor("wv8", [8, 128, 2, C], F8, kind="ExternalInput")
    wo = nc.dram_tensor("wo", [C, C], BF16, kind="ExternalInput")
    vab16 = nc.dram_tensor("vab16", [2, H, HW], BF16, kind="ExternalInput")
    uab = nc.dram_tensor("uab", [2, T], BF16, kind="ExternalInput")
    bo8 = nc.dram_tensor("bo8", [1, C], F8, kind="ExternalInput")
    out = nc.dram_tensor("out", [HW, C], F32, kind="ExternalOutput")

    with tile.TileContext(nc) as tc:
        with (
            tc.tile_pool(name="const", bufs=1) as const,
            tc.tile_pool(name="persist", bufs=1) as persist,
            tc.tile_pool(name="dper", bufs=1, space="PSUM") as dper,
        ):
            ident = const.tile([128, 128], BF16)
            make_identity(nc, ident)
            ones_col = const.tile([T, 1], BF16)
            nc.any.memset(ones_col, 1.0 / SP)

            # persistent state
            wq_sb = persist.tile([128, 4, 2, C], F8)
            kTaug = persist.tile([66, H, T], BF16)
            m_dr = [persist.tile([128, 2, C], F8, name=f"m{i}") for i in range(NKT)]
            prob_dr = [
                [persist.tile([128, 2, NQ], F8, name=f"pb{b}_{i}") for i in range(NKT)]
                for b in range(2)
            ]
            qTaug = [persist.tile([66, H, NQ], BF16, name=f"qa{b}") for b in range(2)]
            hsT_sb = [persist.tile([128, 4, 2, NQ], F8, name=f"ht{b}") for b in range(2)]
            res_sb = [persist.tile([128, 4, C], BF16, name=f"rs{b}") for b in range(3)]
            d_ps = [dper.tile([128, NQ], F32, name=f"dp{b}") for b in range(2)]

            # --- early DMAs (sync engine; one-time loads) ---
            ehsT_sb = persist.tile([128, 8, 2, TPAD], F8)
            for t in range(8):
                nc.sync.dma_start(ehsT_sb[:, t, :, :], ehsT8[t, :, :, :])
            for t in range(4):
                nc.sync.dma_start(wq_sb[:, t, :, :], wq8[t, :, :, :])
            for t in range(4):
                nc.gpsimd.dma_start(hsT_sb[0][:, t, :, :], hsT8[t, :, :, ds(0, NQ)])
            for qj in range(4):
                nc.gpsimd.dma_start(res_sb[0][:, qj, :], hres[ds(128 * qj, 128), :])
            nc.gpsimd.dma_start(qTaug[0][64:66, :, :], vab16[:, :, ds(0, NQ)])
            for h in range(H):
                nc.gpsimd.dma_start(kTaug[64:66, h, :], uab[:, :])
            nc.any.memset(m_dr[BO_KT], 0.0)
            nc.sync.dma_start(m_dr[BO_KT][BO_P : BO_P + 1, BO_I, :], bo8[:, :])
            ones_q8 = const.tile([1, NQ], F8)
            nc.any.memset(ones_q8, 1.0)
            for par in range(2):
                nc.any.memset(prob_dr[par][BO_KT], 0.0)
                nc.sync.dma_start(
                    prob_dr[par][BO_KT][BO_P : BO_P + 1, BO_I, :], ones_q8
                )
            for b in range(2):
                nc.any.memset(d_ps[b], 1.0)

            # qT psum pool lives through stage A and the main loop (2 banks)
            with tc.tile_pool(name="qpl", bufs=2, space="PSUM") as qpl:

                def q_group(k, ij):
                    par = k % 2
                    q_ps = qpl.tile([128, NQ], F32, tag="qps", bufs=2)
                    for t in range(4):
                        nc.tensor.matmul(
                            q_ps,
                            wq_sb[:, t, :, ds(128 * ij, 128)],
                            hsT_sb[par][:, t, :, :],
                            start=(t == 0),
                            stop=(t == 3),
                            perf_mode=DR,
                        )
                    nc.scalar.copy(qTaug[par][0:64, 2 * ij, :], q_ps[0:64, :])
                    nc.scalar.copy(qTaug[par][0:64, 2 * ij + 1, :], q_ps[64:128, :])

                def load_chunk(k):
                    q0 = NQ * k
                    for t in range(4):
                        nc.gpsimd.dma_start(
                            hsT_sb[k % 2][:, t, :, :], hsT8[t, :, :, ds(q0, NQ)]
                        )
                    for qj in range(4):
                        nc.gpsimd.dma_start(
                            res_sb[k % 3][:, qj, :], hres[ds(q0 + 128 * qj, 128), :]
                        )
                    nc.gpsimd.dma_start(
                        qTaug[k % 2][64:66, :, :], vab16[:, :, ds(q0, NQ)]
                    )

                # -------- stage A: k, v (fp8 DR), kTaug, M (bf16) --------
                with (
                    tc.tile_pool(name="sa_sb", bufs=2) as sa_sb,
                    tc.tile_pool(name="sa_w", bufs=3) as sa_w,
                    tc.tile_pool(name="sa_ps", bufs=1, space="PSUM") as sa_ps,
                ):
                    kv_sb = {}
                    for name, w8 in (("k", wk8), ("v", wv8)):
                        kv_ps = sa_ps.tile([TPAD, C], F32, tag="kvps", bufs=1)
                        for t in range(8):
                            wt = sa_w.tile([128, 2, C], F8, tag="wkv")
                            nc.sync.dma_start(wt, w8[t, :, :, :])
                            for nh in range(2):
                                nc.tensor.matmul(
                                    kv_ps[:, ds(512 * nh, 512)],
                                    ehsT_sb[:, t, :, :],
                                    wt[:, :, ds(512 * nh, 512)],
                                    start=(t == 0),
                                    stop=(t == 7),
                                    perf_mode=DR,
                                )
                        kvs = sa_sb.tile([T, C], BF16, tag=f"{name}sb", bufs=1)
                        nc.any.tensor_copy(kvs, kv_ps[0:T, :])
                        kv_sb[name] = kvs
                        if name == "k":
                            # qT for chunk 0 while wv8 streams in
                            for ij in range(C // 128):
                                q_group(0, ij)

                    # kT -> kTaug (per-head 64 rows at partitions 0..64)
                    vT_sb = sa_sb.tile([128, C // 128, T], BF16, bufs=1)
                    for i in range(C // 128):
                        tp = sa_ps.tile([128, T], BF16, tag="tpa", bufs=1)
                        nc.tensor.transpose(
                            tp, kv_sb["k"][:, ds(128 * i, 128)], ident[:T, :T]
                        )
                        nc.scalar.copy(kTaug[0:64, 2 * i, :], tp[0:64, :])
                        nc.scalar.copy(kTaug[0:64, 2 * i + 1, :], tp[64:128, :])
                    for i in range(C // 128):
                        tp = sa_ps.tile([128, T], BF16, tag="tpa", bufs=1)
                        nc.tensor.transpose(
                            tp, kv_sb["v"][:, ds(128 * i, 128)], ident[:T, :T]
                        )
                        nc.any.tensor_copy(vT_sb[:, i, :], tp)

                    # M_h = v_h @ (Wo/SK), cast to fp8 with 1/SP
                    wot = None
                    for h in range(H):
                        i, po = h // 2, (h % 2) * 64
                        if h % 2 == 0:
                            wot = sa_w.tile([128, C], BF16, tag="wot")
                            nc.sync.dma_start(wot, wo[ds(128 * i, 128), :])
                        m_stg = sa_sb.tile([T, C], F8, tag="mstg", bufs=2)
                        for nh in range(2):
                            m_ps = sa_ps.tile([T, 512], F32, tag="mps", bufs=1)
                            nc.tensor.matmul(
                                m_ps,
                                vT_sb[ds(po, 64), i, :],
                                wot[ds(po, 64), ds(512 * nh, 512)],
                                start=True,
                                stop=True,
                            )
                            nc.scalar.activation(
                                m_stg[:, ds(512 * nh, 512)], m_ps,
                                AF.Copy, scale=1.0 / SP,
                            )
                        for (kt, i2, pp, s0, n) in _stack_pieces(T * h, T):
                            nc.gpsimd.dma_start(
                                m_dr[kt][ds(pp, n), i2, :], m_stg[ds(s0, n), :]
                            )

                load_chunk(1)

                # -------- software-pipelined main loop --------
                with (
                    tc.tile_pool(name="work", bufs=2) as work,
                    tc.tile_pool(name="soft", bufs=4) as soft,
                    tc.tile_pool(name="ops", bufs=2, space="PSUM") as ops,
                ):
                    zs = {}

                    def s_head(k, h):
                        par = k % 2
                        sT_ps = ops.tile([T, NQ], F32, tag="sps", bufs=2)
                        nc.tensor.matmul(
                            sT_ps, kTaug[:, h, :], qTaug[par][:, h, :],
                            start=True, stop=True,
                        )
                        z = soft.tile([T, NQ], BF16, tag="z", bufs=6)
                        nc.scalar.activation(z, sT_ps, AF.Exp, scale=ESCALE)
                        zs[h] = z

                    def d_pair(k, p):
                        gp = k * 8 + p
                        dp = d_ps[gp % 2]
                        nc.tensor.matmul(
                            dp[0:1, :], ones_col, zs[2 * p], start=True, stop=True
                        )
                        nc.tensor.matmul(
                            dp[64:65, :], ones_col, zs[2 * p + 1],
                            start=True, stop=True,
                        )
                        rec = soft.tile([128, NQ], F32, tag="rec", bufs=2)
                        nc.vector.reciprocal_approx_fast(rec, dp)
                        rec_bf = soft.tile([128, NQ], BF16, tag="recbf", bufs=2)
                        nc.scalar.copy(rec_bf, rec)
                        par = k % 2
                        for j in range(2):
                            h = 2 * p + j
                            db = soft.tile([T, NQ], BF16, tag="db", bufs=4)
                            nc.gpsimd.dma_start(
                                db,
                                rec_bf[64 * j : 64 * j + 1, :]
                                .unsqueeze(1)
                                .broadcast_to([1, T, NQ]),
                            )
                            pr = soft.tile([T, NQ], F8, tag="pr", bufs=4)
                            nc.vector.tensor_mul(pr, zs[h], db)
                            for (kt, i, pp, s0, n) in _stack_pieces(T * h, T):
                                nc.gpsimd.dma_start(
                                    prob_dr[par][kt][ds(pp, n), i, :],
                                    pr[ds(s0, n), :],
                                )
                            del zs[h]

                    def av_group(k, g):
                        par = k % 2
                        qj, nh = g // 2, g % 2
                        q0 = NQ * k
                        o_ps = ops.tile([128, NQ], F32, tag="ops", bufs=2)
                        for kt in range(NKT):
                            nc.tensor.matmul(
                                o_ps,
                                prob_dr[par][kt][:, :, ds(128 * qj, 128)],
                                m_dr[kt][:, :, ds(512 * nh, 512)],
                                start=(kt == 0),
                                stop=(kt == NKT - 1),
                                perf_mode=DR,
                            )
                        o_sb = work.tile([128, NQ], F32, tag="osb", bufs=3)
                        nc.vector.tensor_add(
                            o_sb, o_ps, res_sb[k % 3][:, qj, ds(512 * nh, 512)]
                        )
                        nc.gpsimd.dma_start(
                            out[ds(q0 + 128 * qj, 128), ds(512 * nh, 512)], o_sb
                        )

                    for ci in range(NCHUNK + 1):
                        for g in range(8):
                            if ci < NCHUNK:
                                s_head(ci, 2 * g)
                                s_head(ci, 2 * g + 1)
                                if ci + 1 < NCHUNK:
                                    q_group(ci + 1, g)
                                if g >= 1:
                                    d_pair(ci, g - 1)
                            if ci >= 1:
                                av_group(ci - 1, g)
                        if ci < NCHUNK:
                            d_pair(ci, 7)
                        if ci + 2 < NCHUNK:
                            load_chunk(ci + 2)

    nc.compile()
    return nc


_NC_CACHE = {}


def get_nc():
    if "nc" not in _NC_CACHE:
        _NC_CACHE["nc"] = build_nc()
    return _NC_CACHE["nc"]


def _bf16(x):
    return np.asarray(x, dtype=ml_dtypes.bfloat16)


def _f8(x):
    return np.clip(np.asarray(x, np.float32), -240.0, 240.0).astype(
        ml_dtypes.float8_e4m3
    )


def _pair4(x, npair):
    """[R, F] -> [npair, 128, 2, F] with row r = 256*t + 128*i + p."""
    r, f = x.shape
    assert r == npair * 256
    return np.ascontiguousarray(
        x.reshape(npair, 2, 128, f).transpose(0, 2, 1, 3)
    )


def make_in_maps(inputs):
    hs = np.asarray(inputs["hidden_states"], dtype=np.float32)
    ehs = np.asarray(inputs["encoder_hidden_states"], dtype=np.float32)
    mask_A = np.asarray(inputs["mask_A"], dtype=np.float32)
    mask_B = np.asarray(inputs["mask_B"], dtype=np.float32)
    Wq = np.asarray(inputs["Wq"], dtype=np.float32)
    Wk = np.asarray(inputs["Wk"], dtype=np.float32)
    Wv = np.asarray(inputs["Wv"], dtype=np.float32)
    Wo = np.asarray(inputs["Wo"], dtype=np.float32)
    bo = np.asarray(inputs["bo"], dtype=np.float32)
    idxA = np.asarray(inputs["token_indices_A"]).astype(np.int64) % T
    idxB = np.asarray(inputs["token_indices_B"]).astype(np.int64) % T

    # suppression aug rows: u = one-hot key-token indicators ("set": B wins),
    # v = (SQ*SK)*(-SUPPRESS)*(1-mask) per query.
    uA = np.zeros(T, np.float32)
    uB = np.zeros(T, np.float32)
    uA[idxA] = 1.0
    uB[idxB] = 1.0
    uA[idxB] = 0.0
    vA = SQ * SK * (-SUPPRESS) * (1.0 - mask_A)
    vB = SQ * SK * (-SUPPRESS) * (1.0 - mask_B)
    vab = np.stack([vA, vB])                      # [2, HW]
    vab16 = _bf16(np.repeat(vab[:, None, :], H, axis=1))  # [2, H, HW]
    uab = _bf16(np.stack([uA, uB]))               # [2, T]

    wq8 = _pair4(_f8(Wq * (SQ / 8.0)), 4)
    wk8 = _pair4(_f8(Wk * SK), 8)
    wv8 = _pair4(_f8(Wv * SK), 8)
    woS = _bf16(Wo / SK)
    bo8 = _f8(bo[None, :])

    in_maps = []
    for b in range(B):
        ehsT = np.zeros((CT, TPAD), np.float32)
        ehsT[:, :T] = ehs[b].T
        in_maps.append(
            {
                "hsT8": _pair4(_f8(hs[b].T), 4),
                "hres": _bf16(hs[b]),
                "wq8": wq8,
                "ehsT8": _pair4(_f8(ehsT), 8),
                "wk8": wk8,
                "wv8": wv8,
                "wo": woS,
                "vab16": vab16,
                "uab": uab,
                "bo8": bo8,
            }
        )
    return in_maps


def kernel(**inputs) -> np.ndarray:
    from concourse.bass_utils import run_bass_kernel_spmd

    nc = get_nc()
    in_maps = make_in_maps(inputs)
    res = run_bass_kernel_spmd(nc, in_maps, core_ids=list(range(B)))
    return np.stack([res.results[b]["out"] for b in range(B)]).astype(np.float32)


# revision 7
# speedup vs baseline: 1.5838x; 1.2910x over previous
"""Trainium2 Bass kernel for nn_DenseAttnProcessor (sparse_attention).

Cross-attention block: q = hs@Wq, k/v = ehs@{Wk,Wv}, per-head
softmax((q k^T)/8 + col_bias) @ v, @Wo + bo + residual.  B=8 batches ->
data-parallel, one batch per NeuronCore (no collectives).

v2 design notes (vs the bf16 baseline):
  * fp8e4 DoubleRow matmuls (K=256 per instruction) for the three big GEMMs:
    qT = Wq^T hsT, k/v = Wk/Wv^T ehsT, and the fused AV out = probsT^T @ M
    (M = v@Wo stacked per head).  Host pre-quantizes/pre-pairs operands.
  * suppression col_bias rides the scores matmul as 2 extra contraction rows
    (K=66): kTaug rows 64/65 = one-hot u_A/u_B over key tokens, qTaug rows
    64/65 = (SQ*SK)*(-20)*(1-mask) per query.  "Set" semantics (B overwrites
    A) is host-encoded in u_A.  No mask multiply on DVE.
  * softmax in the transposed [T, q] orientation: D via K=77 ones-matmul
    pairs into PSUM partitions {0,64}; one batched DVE reciprocal per pair;
    GpSimd partition_broadcast of 1/D; one DVE mul -> fp8 probs (x SP).
  * probs x SP(=4) and M / SP so the AV PSUM is the unscaled attention
    output: epilogue is a single DVE residual add, then DMA out.
  * All steady-state DMAs are triggered from GpSimd (cheap dispatch).
  * Software pipeline per iteration ci: scores/softmax(ci), qT(ci+1),
    AV(ci-1) interleaved in groups of 2 heads so the PE stream stays dense.
"""

import sys

for _p in ("/opt/trn_rl_repo",):
    if _p not in sys.path:
        sys.path.insert(0, _p)

import numpy as np
import ml_dtypes

import concourse.mybir as mybir
import concourse.tile as tile
from concourse import bacc
from concourse.bass import ds
from concourse.masks import make_identity

F32 = mybir.dt.float32
BF16 = mybir.dt.bfloat16
F8 = mybir.dt.float8e4
AF = mybir.ActivationFunctionType
DR = mybir.MatmulPerfMode.DoubleRow

B, HW, C, CT, T, H, D = 8, 4096, 1024, 2048, 77, 16, 64
SUPPRESS = 20.0
NQ = 512                      # q rows per chunk
NCHUNK = HW // NQ             # 8
SQ = 32.0                     # q-side scale folded into Wq (with 1/sqrt(D))
SK = 32.0                     # k-side scale folded into Wk/Wv
SP = 4.0                      # probs scale (probs*SP, M/SP)
ESCALE = 1.0 / (SQ * SK)      # scores psum = SQ*SK*logits
NKT = 5                       # AV K-tile pairs (stack rows 16*77+1 -> 1280)
TPAD = 80                     # ehsT free-dim pad (fp8 DR needs step%16==0)
BO_KT, BO_I, BO_P = 4, 1, 80  # stack row 1232 = 256*4 + 128*1 + 80


def _stack_pieces(g0, nrows):
    """Split stacked rows [g0, g0+nrows) at 128 boundaries.
    Returns [(kt, i, p, s0, n)] with stack row g = 256*kt + 128*i + p."""
    pieces = []
    pos = 0
    while pos < nrows:
        g = g0 + pos
        kt, r = g // 256, g % 256
        i, p = r // 128, r % 128
        n = min(nrows - pos, 128 - p)
        pieces.append((kt, i, p, pos, n))
        pos += n
    return pieces


def build_nc():
    nc = bacc.Bacc("TRN2", target_bir_lowering=False, debug=False)

    hsT8 = nc.dram_tensor("hsT8", [4, 128, 2, HW], F8, kind="ExternalInput")
    hres = nc.dram_tensor("hres", [HW, C], BF16, kind="ExternalInput")
    wq8 = nc.dram_tensor("wq8", [4, 128, 2, C], F8, kind="ExternalInput")
    ehsT8 = nc.dram_tensor("ehsT8", [8, 128, 2, TPAD], F8, kind="ExternalInput")
    wk8 = nc.dram_tensor("wk8", [8, 128, 2, C], F8, kind="ExternalInput")
    wv8 = nc.dram_tensor("wv8", [8, 128, 2, C], F8, kind="ExternalInput")
    wo = nc.dram_tensor("wo", [C, C], BF16, kind="ExternalInput")
    vab16 = nc.dram_tensor("vab16", [2, H, HW], BF16, kind="ExternalInput")
    uab = nc.dram_tensor("uab", [2, T], BF16, kind="ExternalInput")
    bo8 = nc.dram_tensor("bo8", [1, C], F8, kind="ExternalInput")
    out = nc.dram_tensor("out", [HW, C], F32, kind="ExternalOutput")

    with tile.TileContext(nc) as tc:
        with (
            tc.tile_pool(name="const", bufs=1) as const,
            tc.tile_pool(name="persist", bufs=1) as persist,
            tc.tile_pool(name="dper", bufs=1, space="PSUM") as dper,
        ):
            ident = const.tile([128, 128], BF16)
            make_identity(nc, ident)
            ones_col = const.tile([T, 1], BF16)
            nc.any.memset(ones_col, 1.0 / SP)
            ones77 = const.tile([128, T], BF16)
            nc.any.memset(ones77, 1.0)

            # persistent state
            wq_sb = persist.tile([128, 4, 2, C], F8)
            kTaug = persist.tile([66, H, T], BF16)
            m_dr = [persist.tile([128, 2, C], F8, name=f"m{i}") for i in range(NKT)]
            prob_dr = [
                [persist.tile([128, 2, NQ], F8, name=f"pb{b}_{i}") for i in range(NKT)]
                for b in range(2)
            ]
            qTaug = [persist.tile([66, H, NQ], BF16, name=f"qa{b}") for b in range(2)]
            hsT_sb = [persist.tile([128, 4, 2, NQ], F8, name=f"ht{b}") for b in range(2)]
            res_sb = [persist.tile([128, 4, C], BF16, name=f"rs{b}") for b in range(3)]
            d_ps = [dper.tile([128, NQ], F32, name=f"dp{b}") for b in range(1)]

            # --- early DMAs (sync engine; one-time loads) ---
            ehsT_sb = persist.tile([128, 8, 2, TPAD], F8)
            for t in range(8):
                nc.sync.dma_start(ehsT_sb[:, t, :, :], ehsT8[t, :, :, :])
            for t in range(4):
                nc.sync.dma_start(wq_sb[:, t, :, :], wq8[t, :, :, :])
            for t in range(4):
                nc.sync.dma_start(hsT_sb[0][:, t, :, :], hsT8[t, :, :, ds(0, NQ)])
            for qj in range(4):
                nc.sync.dma_start(res_sb[0][:, qj, :], hres[ds(128 * qj, 128), :])
            nc.sync.dma_start(qTaug[0][64:66, :, :], vab16[:, :, ds(0, NQ)])
            for h in range(H):
                nc.sync.dma_start(kTaug[64:66, h, :], uab[:, :])
            nc.any.memset(m_dr[BO_KT], 0.0)
            nc.sync.dma_start(m_dr[BO_KT][BO_P : BO_P + 1, BO_I, :], bo8[:, :])
            ones_q8 = const.tile([1, NQ], F8)
            nc.any.memset(ones_q8, 1.0)
            for par in range(2):
                nc.any.memset(prob_dr[par][BO_KT], 0.0)
                nc.sync.dma_start(
                    prob_dr[par][BO_KT][BO_P : BO_P + 1, BO_I, :], ones_q8
                )
            nc.any.memset(d_ps[0], 1.0)

            # qT psum pool lives through stage A and the main loop (2 banks)
            with tc.tile_pool(name="qpl", bufs=2, space="PSUM") as qpl:

                def q_group(k, ij):
                    par = k % 2
                    q_ps = qpl.tile([128, NQ], F32, tag="qps", bufs=2)
                    for t in range(4):
                        nc.tensor.matmul(
                            q_ps,
                            wq_sb[:, t, :, ds(128 * ij, 128)],
                            hsT_sb[par][:, t, :, :],
                            start=(t == 0),
                            stop=(t == 3),
                            perf_mode=DR,
                        )
                    nc.scalar.copy(qTaug[par][0:64, 2 * ij, :], q_ps[0:64, :])
                    nc.scalar.copy(qTaug[par][0:64, 2 * ij + 1, :], q_ps[64:128, :])

                def load_chunk(k):
                    q0 = NQ * k
                    for t in range(4):
                        nc.sync.dma_start(
                            hsT_sb[k % 2][:, t, :, :], hsT8[t, :, :, ds(q0, NQ)]
                        )
                    for qj in range(4):
                        nc.sync.dma_start(
                            res_sb[k % 3][:, qj, :], hres[ds(q0 + 128 * qj, 128), :]
                        )
                    nc.sync.dma_start(
                        qTaug[k % 2][64:66, :, :], vab16[:, :, ds(q0, NQ)]
                    )

                # -------- stage A: k, v (fp8 DR), kTaug, M (bf16) --------
                with (
                    tc.tile_pool(name="sa_sb", bufs=2) as sa_sb,
                    tc.tile_pool(name="sa_w", bufs=3) as sa_w,
                    tc.tile_pool(name="sa_ps", bufs=1, space="PSUM") as sa_ps,
                ):
                    kv_sb = {}
                    for name, w8 in (("k", wk8), ("v", wv8)):
                        kv_ps = sa_ps.tile([TPAD, C], F32, tag="kvps", bufs=1)
                        for t in range(8):
                            wt = sa_w.tile([128, 2, C], F8, tag="wkv")
                            nc.sync.dma_start(wt, w8[t, :, :, :])
                            for nh in range(2):
                                nc.tensor.matmul(
                                    kv_ps[:, ds(512 * nh, 512)],
                                    ehsT_sb[:, t, :, :],
                                    wt[:, :, ds(512 * nh, 512)],
                                    start=(t == 0),
                                    stop=(t == 7),
                                    perf_mode=DR,
                                )
                        kvs = sa_sb.tile([T, C], BF16, tag=f"{name}sb", bufs=1)
                        nc.any.tensor_copy(kvs, kv_ps[0:T, :])
                        kv_sb[name] = kvs
                        if name == "k":
                            # qT for chunk 0 while wv8 streams in
                            for ij in range(C // 128):
                                q_group(0, ij)

                    # kT -> kTaug (per-head 64 rows at partitions 0..64)
                    vT_sb = sa_sb.tile([128, C // 128, T], BF16, bufs=1)
                    for i in range(C // 128):
                        tp = sa_ps.tile([128, T], BF16, tag="tpa", bufs=1)
                        nc.tensor.transpose(
                            tp, kv_sb["k"][:, ds(128 * i, 128)], ident[:T, :T]
                        )
                        nc.scalar.copy(kTaug[0:64, 2 * i, :], tp[0:64, :])
                        nc.scalar.copy(kTaug[0:64, 2 * i + 1, :], tp[64:128, :])
                    for i in range(C // 128):
                        tp = sa_ps.tile([128, T], BF16, tag="tpa", bufs=1)
                        nc.tensor.transpose(
                            tp, kv_sb["v"][:, ds(128 * i, 128)], ident[:T, :T]
                        )
                        nc.any.tensor_copy(vT_sb[:, i, :], tp)

                    # M_h = v_h @ (Wo/SK), cast to fp8 with 1/SP
                    wot = None
                    for h in range(H):
                        i, po = h // 2, (h % 2) * 64
                        if h % 2 == 0:
                            wot = sa_w.tile([128, C], BF16, tag="wot")
                            nc.sync.dma_start(wot, wo[ds(128 * i, 128), :])
                        m_stg = sa_sb.tile([T, C], F8, tag="mstg", bufs=2)
                        for nh in range(2):
                            m_ps = sa_ps.tile([T, 512], F32, tag="mps", bufs=1)
                            nc.tensor.matmul(
                                m_ps,
                                vT_sb[ds(po, 64), i, :],
                                wot[ds(po, 64), ds(512 * nh, 512)],
                                start=True,
                                stop=True,
                            )
                            nc.scalar.activation(
                                m_stg[:, ds(512 * nh, 512)], m_ps,
                                AF.Copy, scale=1.0 / SP,
                            )
                        for (kt, i2, pp, s0, n) in _stack_pieces(T * h, T):
                            nc.gpsimd.dma_start(
                                m_dr[kt][ds(pp, n), i2, :], m_stg[ds(s0, n), :]
                            )

                load_chunk(1)

                # -------- software-pipelined main loop --------
                with (
                    tc.tile_pool(name="work", bufs=2) as work,
                    tc.tile_pool(name="soft", bufs=4) as soft,
                    tc.tile_pool(name="ops", bufs=2, space="PSUM") as ops,
                ):
                    zs = {}

                    def s_head(k, h):
                        par = k % 2
                        sT_ps = ops.tile([T, NQ], F32, tag="sps", bufs=2)
                        nc.tensor.matmul(
                            sT_ps, kTaug[:, h, :], qTaug[par][:, h, :],
                            start=True, stop=True,
                        )
                        z = soft.tile([T, NQ], BF16, tag="z", bufs=6)
                        nc.scalar.activation(z, sT_ps, AF.Exp, scale=ESCALE)
                        zs[h] = z

                    def d_pair(k, p):
                        dp = d_ps[0]
                        nc.tensor.matmul(
                            dp[0:1, :], ones_col, zs[2 * p], start=True, stop=True
                        )
                        nc.tensor.matmul(
                            dp[64:65, :], ones_col, zs[2 * p + 1],
                            start=True, stop=True,
                        )
                        rec = soft.tile([128, NQ], F32, tag="rec", bufs=2)
                        nc.vector.reciprocal_approx_fast(rec, dp)
                        rec_bf = soft.tile([128, NQ], BF16, tag="recbf", bufs=2)
                        nc.scalar.copy(rec_bf, rec)
                        par = k % 2
                        for j in range(2):
                            h = 2 * p + j
                            db = ops.tile([T, NQ], F32, tag="db", bufs=1)
                            nc.tensor.matmul(
                                db,
                                ones77[64 * j : 64 * j + 1, :],
                                rec_bf[64 * j : 64 * j + 1, :],
                                start=True,
                                stop=True,
                            )
                            pr = soft.tile([T, NQ], F8, tag="pr", bufs=4)
                            nc.vector.tensor_mul(pr, zs[h], db)
                            for (kt, i, pp, s0, n) in _stack_pieces(T * h, T):
                                nc.gpsimd.dma_start(
                                    prob_dr[par][kt][ds(pp, n), i, :],
                                    pr[ds(s0, n), :],
                                )
                            del zs[h]

                    def av_group(k, g):
                        par = k % 2
                        qj, nh = g // 2, g % 2
                        q0 = NQ * k
                        o_ps = ops.tile([128, NQ], F32, tag="ops", bufs=2)
                        for kt in range(NKT):
                            nc.tensor.matmul(
                                o_ps,
                                prob_dr[par][kt][:, :, ds(128 * qj, 128)],
                                m_dr[kt][:, :, ds(512 * nh, 512)],
                                start=(kt == 0),
                                stop=(kt == NKT - 1),
                                perf_mode=DR,
                            )
                        o_sb = work.tile([128, NQ], F32, tag="osb", bufs=3)
                        nc.vector.tensor_add(
                            o_sb, o_ps, res_sb[k % 3][:, qj, ds(512 * nh, 512)]
                        )
                        nc.gpsimd.dma_start(
                            out[ds(q0 + 128 * qj, 128), ds(512 * nh, 512)], o_sb
                        )

                    for ci in range(NCHUNK + 1):
                        for g in range(8):
                            if ci < NCHUNK:
                                s_head(ci, 2 * g)
                                s_head(ci, 2 * g + 1)
                                if ci + 1 < NCHUNK:
                                    q_group(ci + 1, g)
                                if g >= 1:
                                    d_pair(ci, g - 1)
                            if ci >= 1:
                                av_group(ci - 1, g)
                        if ci < NCHUNK:
                            d_pair(ci, 7)
                        if ci + 2 < NCHUNK:
                            load_chunk(ci + 2)

    nc.compile()
    return nc


_NC_CACHE = {}


def get_nc():
    if "nc" not in _NC_CACHE:
        _NC_CACHE["nc"] = build_nc()
    return _NC_CACHE["nc"]


def _bf16(x):
    return np.asarray(x, dtype=ml_dtypes.bfloat16)


def _f8(x):
    return np.clip(np.asarray(x, np.float32), -240.0, 240.0).astype(
        ml_dtypes.float8_e4m3
    )


def _pair4(x, npair):
    """[R, F] -> [npair, 128, 2, F] with row r = 256*t + 128*i + p."""
    r, f = x.shape
    assert r == npair * 256
    return np.ascontiguousarray(
        x.reshape(npair, 2, 128, f).transpose(0, 2, 1, 3)
    )


def make_in_maps(inputs):
    hs = np.asarray(inputs["hidden_states"], dtype=np.float32)
    ehs = np.asarray(inputs["encoder_hidden_states"], dtype=np.float32)
    mask_A = np.asarray(inputs["mask_A"], dtype=np.float32)
    mask_B = np.asarray(inputs["mask_B"], dtype=np.float32)
    Wq = np.asarray(inputs["Wq"], dtype=np.float32)
    Wk = np.asarray(inputs["Wk"], dtype=np.float32)
    Wv = np.asarray(inputs["Wv"], dtype=np.float32)
    Wo = np.asarray(inputs["Wo"], dtype=np.float32)
    bo = np.asarray(inputs["bo"], dtype=np.float32)
    idxA = np.asarray(inputs["token_indices_A"]).astype(np.int64) % T
    idxB = np.asarray(inputs["token_indices_B"]).astype(np.int64) % T

    # suppression aug rows: u = one-hot key-token indicators ("set": B wins),
    # v = (SQ*SK)*(-SUPPRESS)*(1-mask) per query.
    uA = np.zeros(T, np.float32)
    uB = np.zeros(T, np.float32)
    uA[idxA] = 1.0
    uB[idxB] = 1.0
    uA[idxB] = 0.0
    vA = SQ * SK * (-SUPPRESS) * (1.0 - mask_A)
    vB = SQ * SK * (-SUPPRESS) * (1.0 - mask_B)
    vab = np.stack([vA, vB])                      # [2, HW]
    vab16 = _bf16(np.repeat(vab[:, None, :], H, axis=1))  # [2, H, HW]
    uab = _bf16(np.stack([uA, uB]))               # [2, T]

    wq8 = _pair4(_f8(Wq * (SQ / 8.0)), 4)
    wk8 = _pair4(_f8(Wk * SK), 8)
    wv8 = _pair4(_f8(Wv * SK), 8)
    woS = _bf16(Wo / SK)
    bo8 = _f8(bo[None, :])

    in_maps = []
    for b in range(B):
        ehsT = np.zeros((CT, TPAD), np.float32)
        ehsT[:, :T] = ehs[b].T
        in_maps.append(
            {
                "hsT8": _pair4(_f8(hs[b].T), 4),
                "hres": _bf16(hs[b]),
                "wq8": wq8,
                "ehsT8": _pair4(_f8(ehsT), 8),
                "wk8": wk8,
                "wv8": wv8,
                "wo": woS,
                "vab16": vab16,
                "uab": uab,
                "bo8": bo8,
            }
        )
    return in_maps


def kernel(**inputs) -> np.ndarray:
    from concourse.bass_utils import run_bass_kernel_spmd

    nc = get_nc()
    in_maps = make_in_maps(inputs)
    res = run_bass_kernel_spmd(nc, in_maps, core_ids=list(range(B)))
    return np.stack([res.results[b]["out"] for b in range(B)]).astype(np.float32)
